# revision 1
# baseline (speedup 1.0000x reference)
"""Causal-free attention kernel for Trainium2 (8 NeuronCores).

Model (per batch b):
  q/k/v = x @ W{q,k,v}.T + b{q,k,v}            [S, D] -> heads [H, S, 64]
  scoresT[h, sk, sq] = (k_h q_h^T)/8 ; softmax over sk with key-bias
      tw*treatment[b, sk] (confounder bias is constant over the softmax
      axis and cancels)
  out = attn @ v -> merge heads -> @ Wo.T + bo

Sharding: core c -> batch c//4, head-group c%4 (4 heads, 256 dims of the
qkv/out projections). Each core computes its partial of the final
projection; host sums the 4 partials per batch and adds bo.
"""

import numpy as np

B, S, D, H, HD = 2, 2048, 1024, 16, 64
N_CORES = 8
GROUPS = 4          # head-groups per batch
GD = D // GROUPS    # 256 outdims per group
KC = D // 128       # 8 contraction chunks
NT = S // 128       # 16 token chunks
JC = S // 128       # 16 key chunks
PANEL = 1024        # sq panel width
NPAN = S // PANEL   # 2 panels

_CACHE = {}


def _build_nc(do_compile=True, dbg=False, iters=1):
    import concourse.bass as bass  # noqa: F401
    import concourse.mybir as mybir
    import concourse.tile as tile
    from concourse import bacc
    from concourse.masks import make_identity
    from contextlib import ExitStack

    dt = mybir.dt
    f32, f32r, bf16 = dt.float32, dt.float32r, dt.bfloat16
    AF = mybir.ActivationFunctionType

    nc = bacc.Bacc()

    xt = nc.declare_dram_parameter("xt", [D, S], bf16, isOutput=False)
    wq = nc.declare_dram_parameter("wq", [D, GD], bf16, isOutput=False)
    wk = nc.declare_dram_parameter("wk", [D, GD], bf16, isOutput=False)
    wv = nc.declare_dram_parameter("wv", [D, GD], bf16, isOutput=False)
    wo = nc.declare_dram_parameter("wo", [GD, D], bf16, isOutput=False)
    bq = nc.declare_dram_parameter("bq", [128, 2], f32, isOutput=False)
    bk = nc.declare_dram_parameter("bk", [128, 2], f32, isOutput=False)
    bv = nc.declare_dram_parameter("bv", [1, GD], bf16, isOutput=False)
    tb = nc.declare_dram_parameter("tb", [128, JC], f32, isOutput=False)
    out = nc.declare_dram_parameter("out", [S, D], bf16, isOutput=True)
    if dbg:
        dbg_qT = nc.declare_dram_parameter("dbg_qT", [128, S], f32, isOutput=True)
        dbg_kT = nc.declare_dram_parameter("dbg_kT", [128, S], f32, isOutput=True)
        dbg_v = nc.declare_dram_parameter("dbg_v", [128, JC * 130], f32, isOutput=True)
        dbg_op = nc.declare_dram_parameter("dbg_op", [128, NT * 128], f32, isOutput=True)
        dbg_ot = nc.declare_dram_parameter("dbg_ot", [128, S], f32, isOutput=True)

    with tile.TileContext(nc) as tc, ExitStack() as ctx:
        sing = ctx.enter_context(tc.tile_pool(name="sing", bufs=1))
        apool = ctx.enter_context(tc.tile_pool(name="apool", bufs=6))
        dpool = ctx.enter_context(tc.tile_pool(name="dpool", bufs=2))
        psc = ctx.enter_context(tc.tile_pool(name="psc", bufs=2, space="PSUM"))
        ppv = ctx.enter_context(tc.tile_pool(name="ppv", bufs=1, space="PSUM"))
        psm = ctx.enter_context(tc.tile_pool(name="psm", bufs=2, space="PSUM"))
        if iters > 1:
            ctx.enter_context(tc.For_i(
                0, iters, 1,
                hint_engines=(
                    mybir.EngineType.PE,
                    mybir.EngineType.Activation,
                    mybir.EngineType.DVE,
                    mybir.EngineType.SP,
                    mybir.EngineType.Pool,
                )))

        # ---- constants / parameter loads
        ident = sing.tile([128, 128], f32, tag="ident", name="ident")
        make_identity(nc, ident)

        xt3 = xt.rearrange("(c p) t -> c p t", p=128)
        wq3 = wq.rearrange("(c p) m -> c p m", p=128)
        wk3 = wk.rearrange("(c p) m -> c p m", p=128)
        wv3 = wv.rearrange("(c p) m -> c p m", p=128)
        wo3 = wo.rearrange("(c p) m -> c p m", p=128)

        xt_t, wq_t, wk_t, wv_t = [], [], [], []
        for k in range(KC):
            t = sing.tile([128, S], bf16, tag=f"xt{k}", name=f"xt{k}")
            nc.sync.dma_start(t[:], xt3[k])
            xt_t.append(t)
        for k in range(KC):
            for lst, src, nm in ((wq_t, wq3, "wq"), (wk_t, wk3, "wk"),
                                 (wv_t, wv3, "wv")):
                t = sing.tile([128, GD], bf16, tag=f"{nm}{k}", name=f"{nm}{k}")
                nc.sync.dma_start(t[:], src[k])
                lst.append(t)
        wo_t = []
        for k in range(2):
            t = sing.tile([128, D], bf16, tag=f"wo{k}", name=f"wo{k}")
            nc.sync.dma_start(t[:], wo3[k])
            wo_t.append(t)

        bq_sb = sing.tile([128, 2], f32, tag="bq", name="bq")
        nc.sync.dma_start(bq_sb[:], bq[:])
        bk_sb = sing.tile([128, 2], f32, tag="bk", name="bk")
        nc.sync.dma_start(bk_sb[:], bk[:])
        bv_sb = sing.tile([1, GD], bf16, tag="bv", name="bv")
        nc.sync.dma_start(bv_sb[:], bv[:])
        tb_sb = sing.tile([128, JC], f32, tag="tb", name="tb")
        nc.sync.dma_start(tb_sb[:], tb[:])
        ones_sb = sing.tile([1, 128], bf16, tag="ones", name="ones")
        nc.vector.memset(ones_sb[:], 1.0)

        qT = [sing.tile([128, S], bf16, tag=f"qT{p}", name=f"qT{p}") for p in range(2)]
        kT = [sing.tile([128, S], bf16, tag=f"kT{p}", name=f"kT{p}") for p in range(2)]
        v_sb = [sing.tile([128, JC, 130], bf16, tag=f"v{p}", name=f"v{p}") for p in range(2)]
        ot = [sing.tile([128, S], bf16, tag=f"ot{p}", name=f"ot{p}") for p in range(2)]
        op = [sing.tile([128, NT, 128], f32, tag=f"op{p}", name=f"op{p}") for p in range(2)]

        for p in range(2):
            nc.vector.memset(v_sb[p][:, :, 64:65], 1.0)
            nc.vector.memset(v_sb[p][:, :, 129:130], 1.0)

        # ---- phases
        def proj_qk_steps(m, w_t, b_sb, dest, n0=0, n1=S // 512):
            # one m-chunk of the q/k projection, yielding every ~2 matmuls
            for n in range(n0, n1):
                ps = psm.tile([128, 512], f32, tag="sm", name="sm")
                for k in range(KC):
                    nc.tensor.matmul(
                        ps[:],
                        w_t[k][:, m * 128:(m + 1) * 128],
                        xt_t[k][:, n * 512:(n + 1) * 512],
                        start=(k == 0), stop=(k == KC - 1),
                    )
                    if k % 2 == 1:
                        yield
                nc.vector.tensor_scalar_add(
                    dest[:, n * 512:(n + 1) * 512], ps[:], b_sb[:, m:m + 1])
                yield

        def proj_v_steps(pair):
            # one pair of v columns, yielding once per token chunk
            cols = slice(pair * 128, (pair + 1) * 128)
            for mt in range(NT):
                ps = psm.tile([128, 128], f32, tag="sm", name="sm")
                for k in range(KC):
                    nc.tensor.matmul(
                        ps[:],
                        xt_t[k][:, mt * 128:(mt + 1) * 128],
                        wv_t[k][:, cols],
                        start=(k == 0), stop=False,
                    )
                nc.tensor.matmul(
                    ps[:], ones_sb[:], bv_sb[:, cols],
                    start=False, stop=True,
                )
                dst = v_sb[pair][:, mt].rearrange(
                    "p (h c) -> p h c", c=65)[:, :, 0:64]
                src = ps.rearrange("p (h c) -> p h c", c=64)
                nc.vector.tensor_copy(out=dst, in_=src)
                yield

        def out_proj_steps(mt0, mt1):
            for mt in range(mt0, mt1):
                ob = dpool.tile([128, D], bf16, tag="ob", name="ob")
                for n in range(2):
                    pf = psm.tile([128, 512], f32, tag="sm", name="sm")
                    for pair in range(2):
                        nc.tensor.matmul(
                            pf[:],
                            ot[pair][:, mt * 128:(mt + 1) * 128],
                            wo_t[pair][:, n * 512:(n + 1) * 512],
                            start=(pair == 0), stop=(pair == 1),
                        )
                    nc.vector.tensor_copy(
                        out=ob[:, n * 512:(n + 1) * 512], in_=pf[:])
                    nc.sync.dma_start(
                        out[mt * 128:(mt + 1) * 128, n * 512:(n + 1) * 512],
                        ob[:, n * 512:(n + 1) * 512])
                    yield

        def chain(*gens):
            for g in gens:
                yield from g

        def drain(g):
            for _ in g:
                pass

        def attention(pair, hh, panel, filler):
            rows = slice(hh * 64, (hh + 1) * 64)
            pv = ppv.tile([128, PANEL], f32, tag="pv", name="pv")
            # 8 independent 65-wide accumulation slots share banks, so
            # hardware bank-granular start-zeroing cannot be used: zero the
            # tile once and accumulate with start=False.
            nc.vector.memset(pv[:], 0.0)
            for jj in range(0, JC, 2):
                ats = []
                for j in (jj, jj + 1):
                    sc = psc.tile([128, PANEL], f32, tag="sc", name="sc")
                    for n2 in range(PANEL // 512):
                        q0 = panel * PANEL + n2 * 512
                        nc.tensor.matmul(
                            sc[:, n2 * 512:(n2 + 1) * 512],
                            kT[pair][rows, j * 128:(j + 1) * 128],
                            qT[pair][rows, q0:q0 + 512],
                            start=True, stop=True,
                        )
                    at = apool.tile([128, PANEL], bf16, tag="at", name="at")
                    nc.scalar.activation(
                        at[:], sc[:], AF.Exp,
                        bias=tb_sb[:, j:j + 1], scale=0.125)
                    ats.append(at)
                for j, at in zip((jj, jj + 1), ats):
                    for i in range(PANEL // 128):
                        nc.tensor.matmul(
                            pv[:, i * 128:i * 128 + 65],
                            at[:, i * 128:(i + 1) * 128],
                            v_sb[pair][:, j, hh * 65:(hh + 1) * 65],
                            start=False, stop=False,
                            skip_group_check=True,
                        )
                    next(filler, None)
            den = dpool.tile([128, PANEL // 128], f32, tag="den", name="den")
            pv3 = pv.rearrange("p (i c) -> p i c", c=128)
            nc.vector.reciprocal(den[:], pv3[:, :, 64])
            for i in range(PANEL // 128):
                cq = panel * (PANEL // 128) + i
                nc.vector.tensor_scalar_mul(
                    op[pair][:, cq, hh * 64:(hh + 1) * 64],
                    pv[:, i * 128:i * 128 + 64],
                    den[:, i:i + 1])

        def transposes(pair, panel):
            for cq in range(panel * (PANEL // 128), (panel + 1) * (PANEL // 128)):
                pt = psm.tile([128, 128], f32, tag="sm", name="sm")
                nc.tensor.transpose(pt[:], op[pair][:, cq, :], ident[:])
                nc.vector.tensor_copy(
                    out=ot[pair][:, cq * 128:(cq + 1) * 128], in_=pt[:])

        # startup: the minimum needed before pair0-panel0 attention:
        # qT[0] cols 0:1024, all of kT[0], all of pair0 v
        drain(proj_qk_steps(0, wq_t, bq_sb, qT[0], 0, 2))
        drain(proj_qk_steps(0, wk_t, bk_sb, kT[0]))
        drain(proj_v_steps(0))

        # pair0 attention absorbs the rest of the projections as filler
        filler0 = chain(
            proj_qk_steps(0, wq_t, bq_sb, qT[0], 2, 4),
            proj_qk_steps(1, wk_t, bk_sb, kT[1]),
            proj_qk_steps(1, wq_t, bq_sb, qT[1]),
        )
        for panel in range(NPAN):
            for hh in range(2):
                attention(0, hh, panel, filler0)
            transposes(0, panel)
        drain(filler0)

        # pair1-panel0 absorbs the pair1 v projection (few chunks of
        # lookahead so v chunk j lands before the pv matmuls that read it);
        # panel1 absorbs the first half of the output projection.
        filler1 = proj_v_steps(1)
        for _ in range(8):
            next(filler1, None)
        for panel in range(NPAN):
            for hh in range(2):
                attention(1, hh, panel, filler1)
            transposes(1, panel)
            if panel == 0:
                drain(filler1)
                filler1 = out_proj_steps(0, PANEL // 128)
        drain(filler1)
        drain(out_proj_steps(PANEL // 128, NT))

        if dbg:
            nc.sync.dma_start(dbg_qT[:], qT[0][:])
            nc.sync.dma_start(dbg_kT[:], kT[0][:])
            vf = sing.tile([128, JC * 130], f32, tag="vf", name="vf")
            nc.vector.tensor_copy(
                out=vf[:], in_=v_sb[0].rearrange("p a b -> p (a b)"))
            nc.sync.dma_start(dbg_v[:], vf[:])
            nc.sync.dma_start(
                dbg_op[:], op[0].rearrange("p a b -> p (a b)"))
            nc.sync.dma_start(dbg_ot[:], ot[0][:])

    if do_compile:
        nc.compile()
    return nc


def _get_nc():
    if "nc" not in _CACHE:
        _CACHE["nc"] = _build_nc()
    return _CACHE["nc"]


def _host_shard(inputs):
    import ml_dtypes

    bf = ml_dtypes.bfloat16
    f = np.float32
    x = np.asarray(inputs["x"], f)
    treatment = np.asarray(inputs["treatment"], f)
    Wq = np.asarray(inputs["Wq"], f)
    Wk = np.asarray(inputs["Wk"], f)
    Wv = np.asarray(inputs["Wv"], f)
    Wo = np.asarray(inputs["Wo"], f)
    bq = np.asarray(inputs["bq"], f)
    bk = np.asarray(inputs["bk"], f)
    bv = np.asarray(inputs["bv"], f)
    tw = float(np.asarray(inputs["treatment_weight"], f)[0])

    C = np.ascontiguousarray
    in_maps = []
    for c in range(N_CORES):
        b, g = c // GROUPS, c % GROUPS
        o0 = g * GD
        in_maps.append({
            "xt": C(x[b].T).astype(bf),
            "wq": C(Wq[o0:o0 + GD, :].T).astype(bf),
            "wk": C(Wk[o0:o0 + GD, :].T).astype(bf),
            "wv": C(Wv[o0:o0 + GD, :].T).astype(bf),
            "wo": C(Wo[:, o0:o0 + GD].T).astype(bf),
            "bq": C(bq[o0:o0 + GD].reshape(2, 128).T),
            "bk": C(bk[o0:o0 + GD].reshape(2, 128).T),
            "bv": C(bv[o0:o0 + GD].reshape(1, GD)).astype(bf),
            "tb": C((tw * treatment[b]).reshape(JC, 128).T),
        })
    return in_maps


def _host_gather(results, inputs):
    bo = np.asarray(inputs["bo"], np.float32)
    outs = []
    for b in range(B):
        acc = np.zeros((S, D), np.float32)
        for g in range(GROUPS):
            acc += np.asarray(results[b * GROUPS + g]["out"]).astype(np.float32)
        outs.append(acc + bo[None, :])
    return np.stack(outs).astype(np.float32)


def kernel(**inputs):
    from concourse.bass_utils import run_bass_kernel_spmd

    nc = _get_nc()
    in_maps = _host_shard(inputs)
    res = run_bass_kernel_spmd(nc, in_maps, list(range(N_CORES)))
    return _host_gather(res.results, inputs)


def run_traced(inputs, **kw):
    """Test helper: same as kernel() but returns (output, BassKernelResults)."""
    from concourse.bass_utils import run_bass_kernel_spmd

    nc = _get_nc()
    in_maps = _host_shard(inputs)
    res = run_bass_kernel_spmd(nc, in_maps, list(range(N_CORES)), **kw)
    return _host_gather(res.results, inputs), res

